# revision 7
# baseline (speedup 1.0000x reference)
"""Trainium2 Bass kernel for nn_LinearMultiHeadAttention (linear attention,
no softmax).

Math: since k/v are only used in kv = K^T V and everything is linear,
    kv[b,h] = W_k[h] @ G_b @ W_v[h]^T          with G_b = X_b^T X_b
    out_b   = scale * X_b @ M_b^T
    M_b     = W_o @ WqP_b,  WqP_b[h-rows] = kv[b,h]^T @ W_q[h]
This replaces the q/k/v projections entirely: ~87 GFLOP instead of ~142.

Sharding: 8 cores; core c handles batch b=c//2, token half c%2 (2048
tokens) for the output pass. Each core computes the full G_b (both token
halves) itself: duplicating the 4.3 GF Gram half costs ~55us of PE but
avoids a pair AllReduce that measures ~92us exposed (plus the collective
entry barrier and the HAM clock-gate re-throttle the idle gap causes).

All heavy matmuls use float32r (bf16-rate on the PE for free dim >= 256,
~1.5e-4 relative error per matmul). The tiny per-head kv matmuls use fp32.

SBUF pool lifetimes are stack-scheduled on two sides (LIFO per side):
  left:  gv[A..C] g[A..B] xt[A..A] | xts[C..G] mt[D..G] wqp[D..E] wo[E..E]
         | osb[G..G]
  right: kvt[B..D] wk[B..C] wv[B..B] | wq[D..D] (tiny)
"""
import numpy as np

import concourse.bacc as bacc
import concourse.tile as tile
from concourse import mybir
from concourse.bass_utils import run_bass_kernel_spmd

F32R = mybir.dt.float32r
F32 = mybir.dt.float32

N_CORES = 8
B, S, D = 4, 4096, 1024
H, DK = 16, 64
T = (B * S) // N_CORES  # 2048 tokens per core (output half)
NT = T // 128  # 16 token chunks per half
ND = D // 128  # 8 d-blocks
SCALE = 1.0 / 8.0  # 1/sqrt(DK), folded into W_o host-side

_CACHE: dict = {}


def _build(dbg: bool = False):
    nc = bacc.Bacc(
        "TRN2", target_bir_lowering=False, debug=False, num_devices=N_CORES
    )
    # full batch for the Gram matrix; this core's token half, transposed,
    # for the output pass
    xb = nc.dram_tensor("xb", [S, D], F32R, kind="ExternalInput")
    xct = nc.dram_tensor("xct", [D, T], F32R, kind="ExternalInput")
    wvt = nc.dram_tensor("wvt", [D, D], F32R, kind="ExternalInput")
    wkt = nc.dram_tensor("wkt", [D, D], F32R, kind="ExternalInput")
    wq = nc.dram_tensor("wq", [D, D], F32R, kind="ExternalInput")
    wot = nc.dram_tensor("wot", [D, D], F32R, kind="ExternalInput")
    out = nc.dram_tensor("out", [T, D], F32, kind="ExternalOutput")
    if dbg:
        dbg_g = nc.dram_tensor("dbg_g", [D, D], F32, kind="ExternalOutput")
        dbg_gv = nc.dram_tensor("dbg_gv", [D, D], F32, kind="ExternalOutput")
        dbg_kvt = nc.dram_tensor("dbg_kvt", [64, D], F32, kind="ExternalOutput")
        dbg_wqp = nc.dram_tensor("dbg_wqp", [D, D], F32, kind="ExternalOutput")
        dbg_mt = nc.dram_tensor("dbg_mt", [D, D], F32, kind="ExternalOutput")

    with tile.TileContext(nc) as tc:
        # ---- left-side pools for stage A (stack: gv, g, xt) ----------
        p_gv = tc.alloc_tile_pool(name="gv", bufs=1, side="left")
        p_g = tc.alloc_tile_pool(name="g", bufs=1, side="left")
        p_xt = tc.alloc_tile_pool(name="xt", bufs=1, side="left")

        # ---- stage A: G = xb^T @ xb over both token halves -----------
        ps_a = tc.alloc_tile_pool(name="psA", bufs=2, space="PSUM")
        gt = []
        for haf in range(2):
            xt = []
            for i in range(NT):
                t_ = p_xt.tile([128, D], F32R, tag=f"x{i}", name=f"x{haf}_{i}")
                nc.sync.dma_start(
                    t_[:], xb[haf * T + i * 128 : haf * T + (i + 1) * 128, :]
                )
                xt.append(t_)
            for d1 in range(ND):
                pg = ps_a.tile([128, D], F32, tag="pg", name="pg")
                for i in range(NT):
                    st = xt[i][:, d1 * 128 : (d1 + 1) * 128]
                    for j in range(2):
                        nc.tensor.matmul(
                            pg[:, j * 512 : (j + 1) * 512],
                            st,
                            xt[i][:, j * 512 : (j + 1) * 512],
                            start=(i == 0),
                            stop=(i == NT - 1),
                        )
                if haf == 0:
                    g_ = p_g.tile([128, D], F32R, tag=f"g{d1}", name=f"g{d1}")
                    nc.vector.tensor_copy(g_[:], pg[:])
                    gt.append(g_)
                else:
                    nc.vector.tensor_add(gt[d1][:], gt[d1][:].bitcast(F32), pg[:])
                    if dbg:
                        nc.sync.dma_start(
                            dbg_g[d1 * 128 : (d1 + 1) * 128, :],
                            gt[d1][:].bitcast(F32),
                        )
        p_xt.release()
        ps_a.release()

        # ---- stage B: GV[d1, hd] = sum_d2 G[d2, d1]^T W_v^T[d2, hd] --
        p_kv = tc.alloc_tile_pool(name="kv", bufs=1, side="right")
        p_wk = tc.alloc_tile_pool(name="wk", bufs=1, side="right")
        p_wv = tc.alloc_tile_pool(name="wv", bufs=1, side="right")
        ps_b = tc.alloc_tile_pool(name="psB", bufs=2, space="PSUM")
        wv = []
        for i in range(ND):
            t_ = p_wv.tile([128, D], F32R, tag=f"wv{i}", name=f"wv{i}")
            nc.sync.dma_start(t_[:], wvt[i * 128 : (i + 1) * 128, :])
            wv.append(t_)
        # prefetch wk (stage C input) into its already-allocated pool
        wk = []
        for i in range(ND):
            t_ = p_wk.tile([128, D], F32R, tag=f"wk{i}", name=f"wk{i}")
            nc.sync.dma_start(t_[:], wkt[i * 128 : (i + 1) * 128, :])
            wk.append(t_)

        gv = []
        for d1 in range(ND):
            pgv = ps_b.tile([128, D], F32, tag="pgv", name="pgv")
            for d2 in range(ND):
                st = gt[d2][:, d1 * 128 : (d1 + 1) * 128]
                for j in range(2):
                    nc.tensor.matmul(
                        pgv[:, j * 512 : (j + 1) * 512],
                        st,
                        wv[d2][:, j * 512 : (j + 1) * 512],
                        start=(d2 == 0),
                        stop=(d2 == ND - 1),
                    )
            gv_ = p_gv.tile([128, D], F32R, tag=f"gv{d1}", name=f"gv{d1}")
            nc.vector.tensor_copy(gv_[:], pgv[:])
            gv.append(gv_)
            if dbg:
                nc.sync.dma_start(
                    dbg_gv[d1 * 128 : (d1 + 1) * 128, :], gv_[:].bitcast(F32)
                )
        ps_b.release()
        p_wv.release()
        p_g.release()

        # ---- stage C: kv[h] = W_k[h] @ GV[:, h]  (fp32, tiny) --------
        # kv[h][a, b] = sum_d1 W_k^T[d1, h*64+a] * GV[d1, h*64+b]
        ps_c = tc.alloc_tile_pool(name="psC", bufs=2, space="PSUM")
        kvt = p_kv.tile([64, D], F32R, tag="kvt", name="kvt")
        for h in range(H):
            pkv = ps_c.tile([64, 64], F32, tag="pkv", name="pkv")
            for dc in range(ND):
                nc.tensor.matmul(
                    pkv[:],
                    wk[dc][:, h * 64 : (h + 1) * 64].bitcast(F32),
                    gv[dc][:, h * 64 : (h + 1) * 64].bitcast(F32),
                    start=(dc == 0),
                    stop=(dc == ND - 1),
                )
            nc.vector.tensor_copy(kvt[:, h * 64 : (h + 1) * 64], pkv[:])
        if dbg:
            nc.sync.dma_start(dbg_kvt[:], kvt[:].bitcast(F32))
        ps_c.release()
        p_wk.release()
        p_gv.release()

        # xct loads (needed by stage G; DMAs run during D/E)
        p_xts = tc.alloc_tile_pool(name="xts", bufs=1, side="left")
        xts = []
        for i in range(ND):
            t_ = p_xts.tile([128, T], F32R, tag=f"xts{i}", name=f"xts{i}")
            nc.sync.dma_start(t_[:], xct[i * 128 : (i + 1) * 128, :])
            xts.append(t_)

        # ---- stage D: WqP[h][i, d] = sum_j kv[h][j, i] W_q[h*64+j, d]
        p_mt = tc.alloc_tile_pool(name="mt", bufs=1, side="left")
        p_wqp = tc.alloc_tile_pool(name="wqp", bufs=1, side="left")
        p_wq = tc.alloc_tile_pool(name="wqp_w", bufs=1, side="right")
        ps_d = tc.alloc_tile_pool(name="psD", bufs=2, space="PSUM")

        wq_t = []
        for h in range(H):
            t_ = p_wq.tile([64, D], F32R, tag=f"wq{h}", name=f"wq{h}")
            nc.sync.dma_start(t_[:], wq[h * 64 : (h + 1) * 64, :])
            wq_t.append(t_)

        wqp = []
        for i in range(ND):
            wqp.append(
                p_wqp.tile([128, D], F32R, tag=f"wqp{i}", name=f"wqp{i}")
            )
        for h in range(H):
            pwq = ps_d.tile([64, D], F32, tag="pwq", name="pwq")
            for j in range(2):
                nc.tensor.matmul(
                    pwq[:, j * 512 : (j + 1) * 512],
                    kvt[:, h * 64 : (h + 1) * 64],
                    wq_t[h][:, j * 512 : (j + 1) * 512],
                    start=True,
                    stop=True,
                )
            row0 = (h % 2) * 64
            nc.vector.tensor_copy(wqp[h // 2][row0 : row0 + 64, :], pwq[:])
        if dbg:
            for i in range(ND):
                nc.sync.dma_start(
                    dbg_wqp[i * 128 : (i + 1) * 128, :], wqp[i][:].bitcast(F32)
                )
        ps_d.release()
        p_wq.release()
        p_kv.release()

        # ---- stage E: MT[d, dout] = sum_hd WqP[hd, d] WoT[hd, dout] --
        p_wo = tc.alloc_tile_pool(name="wo", bufs=1, side="left")
        ps_e = tc.alloc_tile_pool(name="psE", bufs=2, space="PSUM")

        wo = []
        for i in range(ND):
            t_ = p_wo.tile([128, D], F32R, tag=f"wo{i}", name=f"wo{i}")
            nc.sync.dma_start(t_[:], wot[i * 128 : (i + 1) * 128, :])
            wo.append(t_)

        mt = []
        for db in range(ND):
            pmt = ps_e.tile([128, D], F32, tag="pmt", name="pmt")
            for hc in range(ND):
                st = wqp[hc][:, db * 128 : (db + 1) * 128]
                for j in range(2):
                    nc.tensor.matmul(
                        pmt[:, j * 512 : (j + 1) * 512],
                        st,
                        wo[hc][:, j * 512 : (j + 1) * 512],
                        start=(hc == 0),
                        stop=(hc == ND - 1),
                    )
            mt_ = p_mt.tile([128, D], F32R, tag=f"mt{db}", name=f"mt{db}")
            nc.vector.tensor_copy(mt_[:], pmt[:])
            mt.append(mt_)
            if dbg:
                nc.sync.dma_start(
                    dbg_mt[db * 128 : (db + 1) * 128, :], mt_[:].bitcast(F32)
                )
        ps_e.release()
        p_wo.release()
        p_wqp.release()

        # ---- stage G: out[t, dout] = sum_d xct[d, t] MT[d, dout] -----
        p_osb = tc.alloc_tile_pool(name="osb", bufs=3, side="left")
        ps_g = tc.alloc_tile_pool(name="psG", bufs=2, space="PSUM")
        for tb in range(NT):
            po = ps_g.tile([128, D], F32, tag="po", name="po")
            for dc in range(ND):
                st = xts[dc][:, tb * 128 : (tb + 1) * 128]
                for j in range(2):
                    nc.tensor.matmul(
                        po[:, j * 512 : (j + 1) * 512],
                        st,
                        mt[dc][:, j * 512 : (j + 1) * 512],
                        start=(dc == 0),
                        stop=(dc == ND - 1),
                    )
            osb = p_osb.tile([128, D], F32, tag="osb", name="osb")
            nc.vector.tensor_copy(osb[:], po[:])
            nc.sync.dma_start(out[tb * 128 : (tb + 1) * 128, :], osb[:])
        ps_g.release()
        p_osb.release()
        p_mt.release()
        p_xts.release()

    nc.compile()
    return nc


def _get_nc(dbg: bool = False):
    key = ("nc", dbg)
    if key not in _CACHE:
        _CACHE[key] = _build(dbg)
    return _CACHE[key]


def kernel(x, W_q, W_k, W_v, W_o):
    x = np.ascontiguousarray(np.asarray(x, dtype=np.float32))
    W_q = np.ascontiguousarray(np.asarray(W_q, dtype=np.float32))
    W_k = np.asarray(W_k, dtype=np.float32)
    W_v = np.asarray(W_v, dtype=np.float32)
    W_o = np.asarray(W_o, dtype=np.float32)

    wvt = np.ascontiguousarray(W_v.T)
    wkt = np.ascontiguousarray(W_k.T)
    wot = np.ascontiguousarray((SCALE * W_o).T)

    in_maps = []
    for c in range(N_CORES):
        b, half = divmod(c, 2)
        xct = np.ascontiguousarray(x[b, half * T : (half + 1) * T, :].T)
        in_maps.append(
            {
                "xb": x[b],
                "xct": xct,
                "wvt": wvt,
                "wkt": wkt,
                "wq": W_q,
                "wot": wot,
            }
        )

    nc = _get_nc()
    res = run_bass_kernel_spmd(nc, in_maps, list(range(N_CORES)))

    out = np.empty((B, S, D), dtype=np.float32)
    for c in range(N_CORES):
        b, half = divmod(c, 2)
        out[b, half * T : (half + 1) * T, :] = res.results[c]["out"]
    return out
